# revision 28
# baseline (speedup 1.0000x reference)
"""Multi-head attention forward, distributed over 8 TRN2 NeuronCores.

Sharding: sequence-parallel. Each core owns S/8 = 256 query rows per batch
(512 rows total, batch-major). It computes K^T and V projections for its own
row shard, all-gathers K^T and V across the 8 cores, then computes all 16
heads of attention for its query rows plus the output projection — output
rows are disjoint across cores, so there is no reduce at the end.

Everything on-device stays in the "transposed" layout (feature dim on
partitions) so no transposes are ever needed:
  QT/KT: [d, s]  (d on partitions)    scores^T: [keys, queries]
  V:     [s, d]  (keys on partitions) attn_out^T: [d, queries]
attn_out^T tiles are exactly the lhsT the output projection wants.

Softmax: scores are bounded (|s| < 9 measured), so exp() without
max-subtraction is safe in fp32. The softmax denominator comes for free from
the PV matmul by appending a ones-column to V (M=65). Division by the sum is
a per-free-element scale applied via gpsimd.partition_broadcast + one DVE
multiply on the small [64, 256] attn_out^T tile.

Compute dtype bf16 (fp32 PSUM accumulation): measured end-to-end rel err
~4.6e-3 vs the fp32 reference.
"""

import sys

sys.path.insert(0, "/opt/trn_rl_repo")

import numpy as np
import ml_dtypes

import concourse.bass as bass
import concourse.mybir as mybir
import concourse.tile as tile
from concourse import bacc
from concourse.bass_utils import run_bass_kernel_spmd

R = 8          # cores
B = 2
S = 2048
D = 1024
H = 16
DK = 64
SQ = S // R    # 256 queries per batch per core
ROWS = B * SQ  # 512 rows per core, batch-major
CT = D // 128  # 8 contraction tiles
NKT = S // 128  # 16 key tiles per batch

BF16 = mybir.dt.bfloat16
F32 = mybir.dt.float32
EXP = mybir.ActivationFunctionType.Exp
NP_BF16 = ml_dtypes.bfloat16


def build_graph(debug_taps=False):
    nc = bacc.Bacc(None, target_bir_lowering=False, num_devices=R)

    xT = nc.declare_dram_parameter("xT", [D, ROWS], BF16, isOutput=False)
    wq = nc.declare_dram_parameter("wq", [D, D], BF16, isOutput=False)
    wk = nc.declare_dram_parameter("wk", [D, D], BF16, isOutput=False)
    wv = nc.declare_dram_parameter("wv", [D, D], BF16, isOutput=False)
    wo = nc.declare_dram_parameter("wo", [D, D], BF16, isOutput=False)
    out = nc.declare_dram_parameter("out", [ROWS, D], F32, isOutput=True)
    dbg = {}
    if debug_taps:
        dbg["qt"] = nc.declare_dram_parameter("dbg_qt", [128, CT * ROWS], BF16, isOutput=True)
        dbg["kt"] = nc.declare_dram_parameter("dbg_kt", [R * D, ROWS], BF16, isOutput=True)
        dbg["v"] = nc.declare_dram_parameter("dbg_v", [R * 2 * D, 256], BF16, isOutput=True)
        dbg["at"] = nc.declare_dram_parameter("dbg_at", [128, CT * ROWS], BF16, isOutput=True)

    # collective bounce buffers.
    # KT: [d, s] layout.
    # V: pre-tiled layout — row = (dd*2+hp)*128 + p, col = b*128 + j*64 + c,
    # where the batch-b key index is k = r*256 + j*128 + p and the feature is
    # d = dd*128 + hp*64 + c.  This keeps every DMA ≤3 AP dims.
    cc_in_kt = nc.dram_tensor("cc_in_kt", [D, ROWS], BF16)
    cc_out_kt = nc.dram_tensor("cc_out_kt", [R * D, ROWS], BF16, addr_space="Shared")
    cc_in_v = nc.dram_tensor("cc_in_v", [2 * D, 256], BF16)
    cc_out_v = nc.dram_tensor("cc_out_v", [R * 2 * D, 256], BF16, addr_space="Shared")
    groups = [list(range(R))]

    with tile.TileContext(nc) as tc:
        with tc.tile_pool(name="persist", bufs=1) as pp:
            xT_sb = pp.tile([128, CT, ROWS], BF16)
            wq_sb = pp.tile([128, CT, D], BF16)
            wk_sb = pp.tile([128, CT, D], BF16)
            wv_sb = pp.tile([128, CT, D], BF16)
            wo_sb = pp.tile([128, CT, D], BF16)
            qt_sb = pp.tile([128, CT, ROWS], BF16)
            at_sb = pp.tile([128, CT, ROWS], BF16)
            # double-buffered attention inputs, one buffer pair per dt parity
            kt2 = [pp.tile([128, R, ROWS], BF16, name=f"kt2_{i}") for i in range(2)]
            # V' tiles per (r, b, j) k-tile slot: [data(64) | ones(1)] so the
            # PV matmul (M=65) emits the softmax denominator on partition 64.
            v2e = [pp.tile([128, R, B, 2, 65], BF16, name=f"v2e_{i}") for i in range(2)]
            v2o = [pp.tile([128, R, B, 2, 65], BF16, name=f"v2o_{i}") for i in range(2)]

            nc.sync.dma_start(xT_sb[:], xT.ap().rearrange("(c p) s -> p c s", p=128))
            nc.sync.dma_start(wk_sb[:], wk.ap().rearrange("(c p) d -> p c d", p=128))
            nc.sync.dma_start(wv_sb[:], wv.ap().rearrange("(c p) d -> p c d", p=128))
            nc.sync.dma_start(wq_sb[:], wq.ap().rearrange("(c p) d -> p c d", p=128))
            nc.sync.dma_start(wo_sb[:], wo.ap().rearrange("(c p) d -> p c d", p=128))
            ones_sb = pp.tile([128, 64], F32)
            nc.vector.memset(ones_sb[:], 1.0)
            for i in range(2):
                nc.vector.memset(v2e[i][:, :, :, :, 64:65], 1.0)
                nc.vector.memset(v2o[i][:, :, :, :, 64:65], 1.0)

            # ---- stage A: K^T and V projections for the local shard + AG ----
            with (
                tc.tile_pool(name="proj_ps", bufs=2, space="PSUM") as proj_ps,
                tc.tile_pool(name="stage", bufs=3) as stage,
            ):
                for dt in range(CT):
                    ps = proj_ps.tile([128, ROWS], F32, tag="ps")
                    for ct in range(CT):
                        nc.tensor.matmul(
                            ps[:],
                            wk_sb[:, ct, dt * 128 : (dt + 1) * 128],
                            xT_sb[:, ct, :],
                            start=(ct == 0),
                            stop=(ct == CT - 1),
                        )
                    sb = stage.tile([128, ROWS], BF16, tag="kv")
                    nc.vector.tensor_copy(sb[:], ps[:])
                    nc.sync.dma_start(cc_in_kt[dt * 128 : (dt + 1) * 128, :], sb[:])
                nc.gpsimd.collective_compute(
                    "AllGather",
                    mybir.AluOpType.bypass,
                    replica_groups=groups,
                    ins=[cc_in_kt.ap().opt()],
                    outs=[cc_out_kt.ap().opt()],
                )
                # cc_in_v viewed as [dd, hp, p, b, j, c]
                v_in_view = cc_in_v.ap().rearrange(
                    "(dd hp p) (b j c) -> dd p hp b j c",
                    dd=CT, hp=2, p=128, b=B, j=2, c=DK,
                )
                for st in range(ROWS // 128):  # st = b*2 + j
                    b_, j_ = st // 2, st % 2
                    for nh in range(2):
                        ps = proj_ps.tile([128, 512], F32, tag="ps")
                        for ct in range(CT):
                            nc.tensor.matmul(
                                ps[:],
                                xT_sb[:, ct, st * 128 : (st + 1) * 128],
                                wv_sb[:, ct, nh * 512 : (nh + 1) * 512],
                                start=(ct == 0),
                                stop=(ct == CT - 1),
                            )
                        sb = stage.tile([128, 512], BF16, tag="kv")
                        nc.vector.tensor_copy(sb[:], ps[:])
                        for ddl in range(4):
                            dd = nh * 4 + ddl
                            nc.sync.dma_start(
                                v_in_view[dd, :, :, b_, j_, :],
                                sb[:, ddl * 128 : (ddl + 1) * 128],
                            )
                nc.gpsimd.collective_compute(
                    "AllGather",
                    mybir.AluOpType.bypass,
                    replica_groups=groups,
                    ins=[cc_in_v.ap().opt()],
                    outs=[cc_out_v.ap().opt()],
                )
                # ---- stage B: Q^T projection (overlaps the collectives) ----
                for dt in range(CT):
                    ps = proj_ps.tile([128, ROWS], F32, tag="ps")
                    for ct in range(CT):
                        nc.tensor.matmul(
                            ps[:],
                            wq_sb[:, ct, dt * 128 : (dt + 1) * 128],
                            xT_sb[:, ct, :],
                            start=(ct == 0),
                            stop=(ct == CT - 1),
                        )
                    nc.vector.tensor_copy(qt_sb[:, dt, :], ps[:])

            # views of the gathered K^T / V for per-group DMA
            # cc_out_kt row = r*1024 + dt*128 + p  →  [p, dt, r, s]
            kt_view = cc_out_kt.ap().rearrange(
                "(r dd p) s -> p dd r s", r=R, dd=CT, p=128
            )
            # cc_out_v row = r*2048 + (dd*2+hp)*128 + p, col = b*128 + j*64 + c
            v_view = cc_out_v.ap().rearrange(
                "(r dd hp p) (b j c) -> p dd hp r b j c",
                r=R, dd=CT, hp=2, p=128, b=B, j=2, c=DK,
            )

            # ---- attention: 8 groups of (2 heads × 2 batches) ----
            with (
                tc.tile_pool(name="st_ps", bufs=2, space="PSUM") as st_psp,
                tc.tile_pool(name="at_ps", bufs=4, space="PSUM") as at_psp,
                tc.tile_pool(name="pt", bufs=3) as ptp,
                tc.tile_pool(name="rec", bufs=4) as recp,
            ):
                for dt in range(CT):
                    k2 = kt2[dt % 2]
                    ve = v2e[dt % 2]
                    vo = v2o[dt % 2]
                    nc.sync.dma_start(k2[:], kt_view[:, dt, :, :])
                    for r_ in range(R):
                        for b_ in range(B):
                            nc.sync.dma_start(
                                ve[:, r_, b_, :, 0:64],
                                v_view[:, dt, 0, r_, b_, :, :],
                            )
                            nc.sync.dma_start(
                                vo[:, r_, b_, :, 0:64],
                                v_view[:, dt, 1, r_, b_, :, :],
                            )
                    # one PSUM bank per (batch, head-parity) accumulator:
                    # matmul start=True clears has_written for the WHOLE bank,
                    # so interleaved accumulation groups must not share one.
                    at_ps = [
                        [at_psp.tile([128, SQ], F32, tag="at", name=f"at_{dt}_{b}_{hp}")
                         for hp in range(2)]
                        for b in range(B)
                    ]
                    for kt in range(NKT):
                        rr, jh = kt // 2, kt % 2
                        st4 = st_psp.tile([128, 4 * SQ], F32, tag="st")
                        for hp in range(2):
                            hs = slice(hp * 64, (hp + 1) * 64)
                            for b in range(B):
                                c = hp * 2 + b
                                nc.tensor.matmul(
                                    st4[:, c * SQ : (c + 1) * SQ],
                                    k2[hs, rr, b * SQ + jh * 128 : b * SQ + jh * 128 + 128],
                                    qt_sb[hs, dt, b * SQ : (b + 1) * SQ],
                                    start=True,
                                    stop=True,
                                )
                        pt4 = ptp.tile([128, 4 * SQ], BF16, tag="pt")
                        nc.scalar.activation(pt4[:], st4[:], EXP)
                        for hp in range(2):
                            vt = ve if hp == 0 else vo
                            for b in range(B):
                                c = hp * 2 + b
                                nc.tensor.matmul(
                                    at_ps[b][hp][0:65, :],
                                    vt[:, rr, b, jh, 0:65],
                                    pt4[:, c * SQ : (c + 1) * SQ],
                                    start=(kt == 0),
                                    stop=(kt == NKT - 1),
                                )
                    # normalize by the softmax sums (partition 64 of each
                    # accumulator).  The per-query reciprocal is broadcast
                    # across partitions with a 1-row ones matmul on PE.
                    for b in range(B):
                        bcol = b * SQ
                        for hp in range(2):
                            ps = at_ps[b][hp]
                            rec = recp.tile([128, SQ], F32, tag="rec")
                            # bc shares the score pool's slots (tag "st") so
                            # total PSUM stays within the 8 banks
                            bc_ps = st_psp.tile([64, SQ], F32, tag="st",
                                                name=f"bc_{dt}_{b}_{hp}")
                            bc_sb = recp.tile([64, SQ], F32, tag="bcs")
                            nc.vector.reciprocal(rec[64:65, :], ps[64:65, :])
                            nc.tensor.matmul(
                                bc_ps[:],
                                ones_sb[64:65, :],
                                rec[64:65, :],
                                start=True,
                                stop=True,
                            )
                            nc.vector.tensor_copy(bc_sb[:], bc_ps[:])
                            if hp == 0:
                                nc.vector.tensor_mul(
                                    at_sb[0:64, dt, bcol : bcol + SQ],
                                    ps[0:64, :],
                                    bc_sb[:],
                                )
                            else:
                                shift = recp.tile([64, SQ], BF16, tag="shift")
                                nc.vector.tensor_mul(shift[:], ps[0:64, :], bc_sb[:])
                                nc.sync.dma_start(
                                    at_sb[64:128, dt, bcol : bcol + SQ], shift[:]
                                )

            if debug_taps:
                nc.sync.dma_start(dbg["qt"].ap(), qt_sb[:])
                nc.sync.dma_start(dbg["kt"].ap(), cc_out_kt.ap())
                nc.sync.dma_start(dbg["v"].ap(), cc_out_v.ap())
                nc.sync.dma_start(dbg["at"].ap(), at_sb[:])

            # ---- output projection ----
            with (
                tc.tile_pool(name="o_ps", bufs=2, space="PSUM") as o_psp,
                tc.tile_pool(name="o_sb", bufs=3) as o_sbp,
            ):
                for st in range(ROWS // 128):
                    for nh in range(2):
                        ps = o_psp.tile([128, 512], F32, tag="o")
                        for dt in range(CT):
                            nc.tensor.matmul(
                                ps[:],
                                at_sb[:, dt, st * 128 : (st + 1) * 128],
                                wo_sb[:, dt, nh * 512 : (nh + 1) * 512],
                                start=(dt == 0),
                                stop=(dt == CT - 1),
                            )
                        osb = o_sbp.tile([128, 512], F32, tag="os")
                        nc.vector.tensor_copy(osb[:], ps[:])
                        nc.sync.dma_start(
                            out[st * 128 : (st + 1) * 128, nh * 512 : (nh + 1) * 512],
                            osb[:],
                        )

    nc.compile()
    return nc


_NC = None


def _get_nc():
    global _NC
    if _NC is None:
        _NC = build_graph()
    return _NC


def make_in_maps(x, W_q, W_k, W_v, W_o):
    wq = (np.asarray(W_q, np.float32) * 0.125).astype(NP_BF16)
    wk = np.asarray(W_k, np.float32).astype(NP_BF16)
    wv = np.asarray(W_v, np.float32).astype(NP_BF16)
    wo = np.asarray(W_o, np.float32).astype(NP_BF16)
    x = np.asarray(x, np.float32)
    in_maps = []
    for r in range(R):
        shard = x[:, r * SQ : (r + 1) * SQ, :].reshape(ROWS, D)  # batch-major rows
        xT_r = np.ascontiguousarray(shard.T).astype(NP_BF16)
        in_maps.append({"xT": xT_r, "wq": wq, "wk": wk, "wv": wv, "wo": wo})
    return in_maps


def assemble_out(results):
    full = np.zeros((B, S, D), np.float32)
    for r in range(R):
        o = np.asarray(results[r]["out"], np.float32)
        for b in range(B):
            full[b, r * SQ : (r + 1) * SQ, :] = o[b * SQ : (b + 1) * SQ, :]
    return full


def run(x, W_q, W_k, W_v, W_o, trace=False):
    nc = _get_nc()
    in_maps = make_in_maps(x, W_q, W_k, W_v, W_o)
    res = run_bass_kernel_spmd(nc, in_maps, core_ids=list(range(R)), trace=trace)
    return assemble_out(res.results), res


def kernel(x, W_q, W_k, W_v, W_o):
    out, _ = run(x, W_q, W_k, W_v, W_o)
    return out
